# revision 12
# baseline (speedup 1.0000x reference)
"""Trainium2 Bass kernel for nn_CgpHmmLayer (HMM forward-algorithm log-likelihood).

Problem: batch=64 one-hot sequences [64, 4096, 32], softmax-parameterized HMM
with 128 states; output loglik [64].

Sharding: data-parallel over batch across 8 NeuronCores (8 sequences/core),
A/B/I replicated. No collectives needed.

Algorithm (time-chunked parallel scan):
  The HMM forward operator v -> (A^T v) * e_t is a strong contraction in
  direction-space (Birkhoff; A = softmax(randn) mixes in a couple of steps).
  The 4096-step serial scan is split into K=128 chunks of C=32 steps per
  sequence; each chunk runs as an independent chain started from ones W=2
  steps early ("warmup") so its direction converges, and its contribution
  telescopes exactly as ln(colsum at end) - ln(colsum at window start) on its
  own unnormalized trajectory. Chunk 0 carries the exact initial state I*E_0
  (injected after its dummy warmup). Boundary error is at the bf16 noise
  floor ~2e-5 (validated in numpy vs a float64 reference) << the 2e-2 gate.

  Per core: R = 8 seqs * 128 chunks = 1024 chains as columns, in G=2 groups
  of 512 pipelining in antiphase. Only C + W = 34 sequential steps:
     PE  : psum_g  = A_bf16^T @ alpha_g            [128, 512]
     DVE : alpha_g = psum_g * Ehat[:, slot(j)]     (bf16)
  A constant 32x emission rescale folded into Ehat keeps chains O(1): no
  renormalization at all.

Layout trick (host-side sigma-permute, pure marshalling):
  xT columns are ordered (r, c, b) with r = t mod C, c = t div C: at scan
  step j every chain (c, b) needs E' at time c*C + (j-W), i.e. ONE contiguous
  1024-column "slot" r = j-W. The DVE multiply reads a contiguous slice, and
  emission production streams in lockstep a few slots ahead of the scan --
  no emission prologue; Vector does nothing but the scan multiplies.
  Warmup steps j < W read slot C-W+j shifted one chain position down
  (chain c uses chunk c-1's column); chunk 0 reads garbage there, fixed by
  the exact init injection at j=W.

Emission production (per 1024-col slot):
  4 row-tiled CONCURRENT PE matmuls (K=32 strips at partition offsets
  0/32/64/96; expB replicated 4x and xT pre-stripped on host) write the four
  256-col quarters of one [128,1024] 2-bank PSUM tile; ScalarE then does a
  single FD=1024 scale-copy (r32 emission-softmax denominator folded in) to
  bf16 ehat. Row tiling packs the K=32 matmuls 4-deep in the 128x128 array.

  A dense burst of dummy matmuls at kernel start (during the DMA/param wait)
  tries to keep the PE HAM clock-gate from idling at 1.2 GHz.

  loglik[b] = sum_c ln(sb[c,b]) - sum_{c>=1} ln(sa[c,b])
              - T*ln(32) - ln(sum(expI))        (final combine on host)
"""
import math
from contextlib import ExitStack

import numpy as np

B, T, ALPH, S = 64, 4096, 32, 128
NC = 8
NB = B // NC          # sequences per core

K = 128               # time chunks per sequence
W = 1                 # warmup steps per chain
C = T // K            # chunk length (steps per chain window)
N = C + W             # total scan steps
G = 2                 # pipeline groups
CPG = K // G          # chunks per group
FD = CPG * NB         # columns per group (512)
SLOT = K * NB         # columns per time-slot (1024)
QC = SLOT // 4        # columns per production strip (256)

PRE_SLOTS = 3         # slots produced before the scan starts
N_WARM_MM = 88        # dummy PE matmuls to trip the HAM clock-gate warm

_COMPILED = None


def _kernel_body(tc, xT, aL, bL, bLT, iL, out):
    import concourse.bass as bass
    from concourse import mybir

    nc = tc.nc
    f32 = mybir.dt.float32
    bf16 = mybir.dt.bfloat16
    AX = mybir.AxisListType
    OP = mybir.AluOpType
    AF = mybir.ActivationFunctionType

    with ExitStack() as ctx:
        singles = ctx.enter_context(tc.tile_pool(name="singles", bufs=1))
        eppool = ctx.enter_context(tc.tile_pool(name="eppool", bufs=2, space="PSUM"))
        gpsum0 = ctx.enter_context(tc.tile_pool(name="gpsum0", bufs=2, space="PSUM"))
        gpsum1 = ctx.enter_context(tc.tile_pool(name="gpsum1", bufs=2, space="PSUM"))
        apool0 = ctx.enter_context(tc.tile_pool(name="apool0", bufs=2))
        apool1 = ctx.enter_context(tc.tile_pool(name="apool1", bufs=2))

        gpsum = (gpsum0, gpsum1)
        apool = (apool0, apool1)

        # ---------------- PE warm-up burst (runs during DMA/param wait) -----
        # Targets a corner of the first gpsum0 ring buffer; the scan's own
        # WAW ordering makes this safe.
        wdum = singles.tile([S, 64], bf16)
        nc.vector.memset(wdum[:], 0.25)
        burst_ps = gpsum0.tile([S, FD], f32, tag="p0")
        for _ in range(N_WARM_MM):
            nc.tensor.matmul(
                burst_ps[0:64, 0:64], wdum[:, 0:64], wdum[:], start=True, stop=True
            )

        # ---------------- parameter prep ----------------
        aL_sb = singles.tile([S, S], f32)
        nc.sync.dma_start(aL_sb[:], aL)
        bN_sb = singles.tile([ALPH, S], f32)
        nc.sync.dma_start(bN_sb[:], bL)
        bT_sb = singles.tile([S, ALPH], f32)
        nc.sync.dma_start(bT_sb[:], bLT)
        iL_sb = singles.tile([S, 1], f32)
        nc.sync.dma_start(
            iL_sb[:], bass.AP(tensor=iL.tensor, offset=0, ap=[[1, S], [S, 1]])
        )

        # expB = exp(B_logits) bf16 (emission lhsT); r32[s] = 32/sum_a expB[a,s]
        # via the transposed copy (free-dim reduce). B_logits ~ N(0,1): raw
        # exp() is safe.
        expB = singles.tile([ALPH, S], bf16)
        nc.scalar.activation(expB[:], bN_sb[:], AF.Exp)
        expBT = singles.tile([S, ALPH], f32)
        nc.scalar.activation(expBT[:], bT_sb[:], AF.Exp)
        bsum = singles.tile([S, 1], f32)
        nc.vector.tensor_reduce(bsum[:], expBT[:], axis=AX.X, op=OP.add)
        r32 = singles.tile([S, 1], f32)
        nc.vector.reciprocal(r32[:], bsum[:])
        nc.vector.tensor_scalar_mul(r32[:], r32[:], 32.0)

        # A = softmax(rows of A_logits), stored bf16 (scan stationary operand)
        rowmax = singles.tile([S, 1], f32)
        nc.vector.tensor_reduce(rowmax[:], aL_sb[:], axis=AX.X, op=OP.max)
        negmax = singles.tile([S, 1], f32)
        nc.vector.tensor_scalar_mul(negmax[:], rowmax[:], -1.0)
        expA = singles.tile([S, S], f32)
        nc.scalar.activation(expA[:], aL_sb[:], AF.Exp, bias=negmax[:], scale=1.0)
        rowsum = singles.tile([S, 1], f32)
        nc.vector.tensor_reduce(rowsum[:], expA[:], axis=AX.X, op=OP.add)
        rrow = singles.tile([S, 1], f32)
        nc.vector.reciprocal(rrow[:], rowsum[:])
        A_sb = singles.tile([S, S], bf16)
        nc.vector.tensor_scalar_mul(A_sb[:], expA[:], rrow[:])

        # expI (fp32, for the exact chunk-0 init)
        expI = singles.tile([S, 1], f32)
        nc.scalar.activation(expI[:], iL_sb[:], AF.Exp)

        ones_col = singles.tile([S, 1], bf16)
        nc.vector.memset(ones_col[:], 1.0)

        # ---------------- input DMA (slot-production order) ----------------
        xT_sb = singles.tile([ALPH, T * NB], bf16)
        # first transfer: just slot C-1 (the scan's step-0 operand), then the
        # remaining range in 7 slices + the leftover of the last region
        spans = [((C - 1) * SLOT, C * SLOT)]
        dma_w = (T * NB) // 8
        spans += [(i * dma_w, (i + 1) * dma_w) for i in range(7)]
        spans += [(7 * dma_w, (C - 1) * SLOT)]
        for lo, hi in spans:
            nc.gpsimd.dma_start(xT_sb[:, lo:hi], xT[:, lo:hi])

        # ---------------- emission (streamed by slot) ----------------
        ehat = singles.tile([S, T * NB], bf16)

        def produce_slot(r):
            # two 512-col matmuls into the two banks of one [S,1024] PSUM
            # tile; a single FD=1024 ScalarE scale-copy drains it to bf16
            ep = eppool.tile([S, SLOT], f32, tag="eps")
            for h in range(2):
                lo = r * SLOT + h * FD
                nc.tensor.matmul(
                    ep[:, h * FD : (h + 1) * FD],
                    expB[:],
                    xT_sb[:, lo : lo + FD],
                    start=True,
                    stop=True,
                )
            nc.scalar.activation(
                ehat[:, r * SLOT : (r + 1) * SLOT], ep[:], AF.Copy, scale=r32[:]
            )

        # slots consumed: j<W -> slot C-W+j (shifted); j>=W -> slot j-W.
        # Dummy-mm bridges keep PE activity solid while production waits on
        # the ScalarE PSUM drain: a fully-idle >3.4us window would drop the
        # HAM clock gate back to 1.2 GHz (warm persists through short gaps).
        for r in list(range(C - W, C)) + list(range(PRE_SLOTS - W)):
            produce_slot(r)
            for _ in range(24):
                nc.tensor.matmul(
                    burst_ps[0:64, 0:64], wdum[:, 0:64], wdum[:], start=True, stop=True
                )

        # ---------------- the scan ----------------
        alpha = []
        for g in range(G):
            a0 = apool[g].tile([S, FD], bf16, tag=f"a{g}")
            nc.vector.memset(a0[:], 1.0)
            alpha.append(a0)

        sums_sb = singles.tile([1, 4 * FD], f32)
        ps_now = [None, None]

        def capture_sums(slot):
            # colsum via ones-matmul into row 0 of the just-consumed scan
            # psum buffer (dead until its ring slot recycles two steps later)
            for g in range(G):
                nc.tensor.matmul(
                    ps_now[g][0:1, :], ones_col[:], alpha[g][:], start=True, stop=True
                )
                o = (2 * slot + g) * FD
                nc.scalar.activation(sums_sb[:, o : o + FD], ps_now[g][0:1, :], AF.Copy)

        for j in range(N):
            base = (C - W + j) * SLOT - NB if j < W else (j - W) * SLOT
            for g in range(G):
                ps = gpsum[g].tile([S, FD], f32, tag=f"p{g}")
                nc.tensor.matmul(ps[:], A_sb[:], alpha[g][:], start=True, stop=True)
                anew = apool[g].tile([S, FD], bf16, tag=f"a{g}")
                nc.vector.tensor_mul(
                    anew[:], ps[:], ehat[:, base + g * FD : base + (g + 1) * FD]
                )
                alpha[g] = anew
                ps_now[g] = ps
            s = j + PRE_SLOTS - W
            if s < C - W:
                produce_slot(s)
            if j == W - 1:
                capture_sums(0)  # s_a: chain state at time c*C - 1
            if j == W:
                # exact chunk-0 init: alpha(b, c=0) := expI * E'_0 (t=0 is
                # slot 0 position 0 -> ehat cols [0, NB))
                nc.vector.tensor_scalar_mul(alpha[0][:, 0:NB], ehat[:, 0:NB], expI[:])

        capture_sums(1)  # s_b: chain state at time (c+1)*C - 1

        nc.sync.dma_start(out, sums_sb[:])


def _build():
    import concourse.tile as tile
    from concourse import bacc, mybir

    f32 = mybir.dt.float32
    bf16 = mybir.dt.bfloat16

    nc = bacc.Bacc("TRN2", target_bir_lowering=False, debug=False)
    xT_t = nc.dram_tensor("xT", [ALPH, T * NB], bf16, kind="ExternalInput")
    aL_t = nc.dram_tensor("A_logits", [S, S], f32, kind="ExternalInput")
    bL_t = nc.dram_tensor("B_logits", [ALPH, S], f32, kind="ExternalInput")
    bLT_t = nc.dram_tensor("B_logitsT", [S, ALPH], f32, kind="ExternalInput")
    iL_t = nc.dram_tensor("I_logits", [S], f32, kind="ExternalInput")
    out_t = nc.dram_tensor("sums", [4 * FD], f32, kind="ExternalOutput")

    with tile.TileContext(nc) as tc:
        _kernel_body(
            tc, xT_t.ap(), aL_t.ap(), bL_t.ap(), bLT_t.ap(), iL_t.ap(), out_t.ap()
        )
    nc.compile()
    return nc


def _shard_inputs(inputs, A_logits, B_logits, I_logits):
    import ml_dtypes

    in_maps = []
    bF = np.ascontiguousarray(B_logits, dtype=np.float32)
    bT = np.ascontiguousarray(B_logits.T, dtype=np.float32)
    aF = np.ascontiguousarray(A_logits, dtype=np.float32)
    iF = np.ascontiguousarray(I_logits, dtype=np.float32)
    for c in range(NC):
        xc = inputs[c * NB : (c + 1) * NB]                  # [NB, T, 32]
        # sigma-permute: column (r, chunk, b) with r = t mod C, chunk = t//C
        xs = xc.reshape(NB, K, C, ALPH).transpose(3, 2, 1, 0)   # [A, C, K, NB]
        xTc = np.ascontiguousarray(xs.reshape(ALPH, T * NB)).astype(
            ml_dtypes.bfloat16
        )
        in_maps.append(
            {
                "xT": xTc,
                "A_logits": aF,
                "B_logits": bF,
                "B_logitsT": bT,
                "I_logits": iF,
            }
        )
    return in_maps


def kernel(inputs, A_logits, B_logits, I_logits):
    from concourse.bass_utils import run_bass_kernel_spmd

    global _COMPILED
    if _COMPILED is None:
        _COMPILED = _build()

    in_maps = _shard_inputs(inputs, A_logits, B_logits, I_logits)
    res = run_bass_kernel_spmd(_COMPILED, in_maps, list(range(NC)))

    ln_corr = T * math.log(32.0) + math.log(
        np.exp(I_logits.astype(np.float64)).sum()
    )
    out = np.empty(B, np.float64)
    for c in range(NC):
        sums = np.asarray(res.results[c]["sums"], dtype=np.float64)
        sa = sums[0 : 2 * FD].reshape(K, NB)        # [chunk, seq-in-core]
        sb = sums[2 * FD : 4 * FD].reshape(K, NB)
        ll = np.log(sb).sum(0) - np.log(sa[1:]).sum(0) - ln_corr
        out[c * NB : (c + 1) * NB] = ll
    return out.astype(np.float32)


# revision 14
# speedup vs baseline: 1.0738x; 1.0738x over previous
"""Trainium2 Bass kernel for nn_CgpHmmLayer (HMM forward-algorithm log-likelihood).

Problem: batch=64 one-hot sequences [64, 4096, 32], softmax-parameterized HMM
with 128 states; output loglik [64].

Sharding: data-parallel over batch across 8 NeuronCores (8 sequences/core),
A/B/I replicated. No collectives needed.

Algorithm (time-chunked parallel scan):
  The HMM forward operator v -> (A^T v) * e_t is a strong contraction in
  direction-space (Birkhoff; A = softmax(randn) mixes in a couple of steps).
  The 4096-step serial scan is split into K=128 chunks of C=32 steps per
  sequence; each chunk runs as an independent chain started from ones W=2
  steps early ("warmup") so its direction converges, and its contribution
  telescopes exactly as ln(colsum at end) - ln(colsum at window start) on its
  own unnormalized trajectory. Chunk 0 carries the exact initial state I*E_0
  (injected after its dummy warmup). Boundary error is at the bf16 noise
  floor ~2e-5 (validated in numpy vs a float64 reference) << the 2e-2 gate.

  Per core: R = 8 seqs * 128 chunks = 1024 chains as columns, in G=2 groups
  of 512 pipelining in antiphase. Only C + W = 34 sequential steps:
     PE  : psum_g  = A_bf16^T @ alpha_g            [128, 512]
     DVE : alpha_g = psum_g * Ehat[:, slot(j)]     (bf16)
  A constant 32x emission rescale folded into Ehat keeps chains O(1): no
  renormalization at all.

Layout trick (host-side sigma-permute, pure marshalling):
  xT columns are ordered (r, c, b) with r = t mod C, c = t div C: at scan
  step j every chain (c, b) needs E' at time c*C + (j-W), i.e. ONE contiguous
  1024-column "slot" r = j-W. The DVE multiply reads a contiguous slice, and
  emission production streams in lockstep a few slots ahead of the scan --
  no emission prologue; Vector does nothing but the scan multiplies.
  Warmup steps j < W read slot C-W+j shifted one chain position down
  (chain c uses chunk c-1's column); chunk 0 reads garbage there, fixed by
  the exact init injection at j=W.

Emission production (per 1024-col slot):
  4 row-tiled CONCURRENT PE matmuls (K=32 strips at partition offsets
  0/32/64/96; expB replicated 4x and xT pre-stripped on host) write the four
  256-col quarters of one [128,1024] 2-bank PSUM tile; ScalarE then does a
  single FD=1024 scale-copy (r32 emission-softmax denominator folded in) to
  bf16 ehat. Row tiling packs the K=32 matmuls 4-deep in the 128x128 array.

  A dense burst of dummy matmuls at kernel start (during the DMA/param wait)
  tries to keep the PE HAM clock-gate from idling at 1.2 GHz.

  loglik[b] = sum_c ln(sb[c,b]) - sum_{c>=1} ln(sa[c,b])
              - T*ln(32) - ln(sum(expI))        (final combine on host)
"""
import math
from contextlib import ExitStack

import numpy as np

B, T, ALPH, S = 64, 4096, 32, 128
NC = 8
NB = B // NC          # sequences per core

K = 128               # time chunks per sequence
W = 1                 # warmup steps per chain
C = T // K            # chunk length (steps per chain window)
N = C + W             # total scan steps
G = 2                 # pipeline groups
CPG = K // G          # chunks per group
FD = CPG * NB         # columns per group (512)
SLOT = K * NB         # columns per time-slot (1024)
QC = SLOT // 4        # columns per production strip (256)

PRE_SLOTS = 3         # slots produced before the scan starts
N_WARM_MM = 88        # dummy PE matmuls to trip the HAM clock-gate warm

_COMPILED = None


def _kernel_body(tc, xT, aL, bL, bLT, iL, out):
    import concourse.bass as bass
    from concourse import mybir

    nc = tc.nc
    f32 = mybir.dt.float32
    bf16 = mybir.dt.bfloat16
    AX = mybir.AxisListType
    OP = mybir.AluOpType
    AF = mybir.ActivationFunctionType

    with ExitStack() as ctx:
        singles = ctx.enter_context(tc.tile_pool(name="singles", bufs=1))
        eppool = ctx.enter_context(tc.tile_pool(name="eppool", bufs=2, space="PSUM"))
        gpsum0 = ctx.enter_context(tc.tile_pool(name="gpsum0", bufs=2, space="PSUM"))
        gpsum1 = ctx.enter_context(tc.tile_pool(name="gpsum1", bufs=2, space="PSUM"))
        apool0 = ctx.enter_context(tc.tile_pool(name="apool0", bufs=2))
        apool1 = ctx.enter_context(tc.tile_pool(name="apool1", bufs=2))

        gpsum = (gpsum0, gpsum1)
        apool = (apool0, apool1)

        # ---------------- PE warm-up burst (runs during DMA/param wait) -----
        # Targets a corner of the first gpsum0 ring buffer; the scan's own
        # WAW ordering makes this safe.
        wdum = singles.tile([S, 64], bf16)
        nc.vector.memset(wdum[:], 0.25)
        burst_ps = gpsum0.tile([S, FD], f32, tag="p0")
        for _ in range(N_WARM_MM):
            nc.tensor.matmul(
                burst_ps[0:64, 0:64], wdum[:, 0:64], wdum[:], start=True, stop=True
            )

        # ---------------- parameter prep ----------------
        # Each engine runs its ops strictly in FIFO order, so everything is
        # emitted in dependency-readiness order: B-path DMAs/exp first (the
        # emission pipeline needs expB+r32 ASAP; a stalled op at the scalar
        # FIFO head would block the emission copies queued behind it), then
        # the A softmax (only gates the first scan matmul), then expI.
        bN_sb = singles.tile([ALPH, S], f32)
        nc.sync.dma_start(bN_sb[:], bL)
        bT_sb = singles.tile([S, ALPH], f32)
        nc.sync.dma_start(bT_sb[:], bLT)
        aL_sb = singles.tile([S, S], f32)
        nc.sync.dma_start(aL_sb[:], aL)
        iL_sb = singles.tile([S, 1], f32)
        nc.sync.dma_start(
            iL_sb[:], bass.AP(tensor=iL.tensor, offset=0, ap=[[1, S], [S, 1]])
        )

        # ---------------- input DMA (slot-production order) ----------------
        xT_sb = singles.tile([ALPH, T * NB], bf16)
        # first transfer: just slot C-1 (the scan's step-0 operand), then the
        # remaining range in 7 slices + the leftover of the last region
        spans = [((C - 1) * SLOT, C * SLOT)]
        dma_w = (T * NB) // 8
        spans += [(i * dma_w, (i + 1) * dma_w) for i in range(7)]
        spans += [(7 * dma_w, (C - 1) * SLOT)]
        for lo, hi in spans:
            nc.gpsimd.dma_start(xT_sb[:, lo:hi], xT[:, lo:hi])

        # expB = exp(B_logits) bf16 (emission lhsT); r32[s] = 32/sum_a expB[a,s]
        # via the transposed copy (free-dim reduce). B_logits ~ N(0,1): raw
        # exp() is safe.
        expB = singles.tile([ALPH, S], bf16)
        nc.scalar.activation(expB[:], bN_sb[:], AF.Exp)
        expBT = singles.tile([S, ALPH], f32)
        nc.scalar.activation(expBT[:], bT_sb[:], AF.Exp)
        bsum = singles.tile([S, 1], f32)
        nc.vector.tensor_reduce(bsum[:], expBT[:], axis=AX.X, op=OP.add)
        r32 = singles.tile([S, 1], f32)
        nc.vector.reciprocal(r32[:], bsum[:])
        nc.vector.tensor_scalar_mul(r32[:], r32[:], 32.0)

        # A = softmax(rows of A_logits), stored bf16 (scan stationary operand)
        rowmax = singles.tile([S, 1], f32)
        nc.vector.tensor_reduce(rowmax[:], aL_sb[:], axis=AX.X, op=OP.max)
        negmax = singles.tile([S, 1], f32)
        nc.vector.tensor_scalar_mul(negmax[:], rowmax[:], -1.0)
        expA = singles.tile([S, S], f32)
        nc.scalar.activation(expA[:], aL_sb[:], AF.Exp, bias=negmax[:], scale=1.0)
        # expI (fp32, for the exact chunk-0 init)
        expI = singles.tile([S, 1], f32)
        nc.scalar.activation(expI[:], iL_sb[:], AF.Exp)
        rowsum = singles.tile([S, 1], f32)
        nc.vector.tensor_reduce(rowsum[:], expA[:], axis=AX.X, op=OP.add)
        rrow = singles.tile([S, 1], f32)
        nc.vector.reciprocal(rrow[:], rowsum[:])
        A_sb = singles.tile([S, S], bf16)
        nc.vector.tensor_scalar_mul(A_sb[:], expA[:], rrow[:])

        ones_col = singles.tile([S, 1], bf16)
        nc.vector.memset(ones_col[:], 1.0)

        # ---------------- emission (streamed by slot) ----------------
        ehat = singles.tile([S, T * NB], bf16)

        def produce_slot(r):
            # two 512-col matmuls into the two banks of one [S,1024] PSUM
            # tile; a single FD=1024 ScalarE scale-copy drains it to bf16
            ep = eppool.tile([S, SLOT], f32, tag="eps")
            for h in range(2):
                lo = r * SLOT + h * FD
                nc.tensor.matmul(
                    ep[:, h * FD : (h + 1) * FD],
                    expB[:],
                    xT_sb[:, lo : lo + FD],
                    start=True,
                    stop=True,
                )
            nc.scalar.activation(
                ehat[:, r * SLOT : (r + 1) * SLOT], ep[:], AF.Copy, scale=r32[:]
            )

        # slots consumed: j<W -> slot C-W+j (shifted); j>=W -> slot j-W.
        # Dummy-mm bridges keep PE activity solid while production waits on
        # the ScalarE PSUM drain: a fully-idle >3.4us window would drop the
        # HAM clock gate back to 1.2 GHz (warm persists through short gaps).
        for r in list(range(C - W, C)) + list(range(PRE_SLOTS - W)):
            produce_slot(r)
            for _ in range(10):
                nc.tensor.matmul(
                    burst_ps[0:64, 0:64], wdum[:, 0:64], wdum[:], start=True, stop=True
                )

        # ---------------- the scan ----------------
        alpha = []
        for g in range(G):
            a0 = apool[g].tile([S, FD], bf16, tag=f"a{g}")
            nc.vector.memset(a0[:], 1.0)
            alpha.append(a0)

        sums_sb = singles.tile([1, 4 * FD], f32)
        ps_now = [None, None]

        def capture_sums(slot):
            # colsum via ones-matmul into row 0 of the just-consumed scan
            # psum buffer (dead until its ring slot recycles two steps later)
            for g in range(G):
                nc.tensor.matmul(
                    ps_now[g][0:1, :], ones_col[:], alpha[g][:], start=True, stop=True
                )
                o = (2 * slot + g) * FD
                nc.scalar.activation(sums_sb[:, o : o + FD], ps_now[g][0:1, :], AF.Copy)

        for j in range(N):
            base = (C - W + j) * SLOT - NB if j < W else (j - W) * SLOT
            for g in range(G):
                ps = gpsum[g].tile([S, FD], f32, tag=f"p{g}")
                nc.tensor.matmul(ps[:], A_sb[:], alpha[g][:], start=True, stop=True)
                anew = apool[g].tile([S, FD], bf16, tag=f"a{g}")
                nc.vector.tensor_mul(
                    anew[:], ps[:], ehat[:, base + g * FD : base + (g + 1) * FD]
                )
                alpha[g] = anew
                ps_now[g] = ps
            s = j + PRE_SLOTS - W
            if s < C - W:
                produce_slot(s)
            if j == W - 1:
                capture_sums(0)  # s_a: chain state at time c*C - 1
            if j == W:
                # exact chunk-0 init: alpha(b, c=0) := expI * E'_0 (t=0 is
                # slot 0 position 0 -> ehat cols [0, NB))
                nc.vector.tensor_scalar_mul(alpha[0][:, 0:NB], ehat[:, 0:NB], expI[:])

        capture_sums(1)  # s_b: chain state at time (c+1)*C - 1

        nc.sync.dma_start(out, sums_sb[:])


def _build():
    import concourse.tile as tile
    from concourse import bacc, mybir

    f32 = mybir.dt.float32
    bf16 = mybir.dt.bfloat16

    nc = bacc.Bacc("TRN2", target_bir_lowering=False, debug=False)
    xT_t = nc.dram_tensor("xT", [ALPH, T * NB], bf16, kind="ExternalInput")
    aL_t = nc.dram_tensor("A_logits", [S, S], f32, kind="ExternalInput")
    bL_t = nc.dram_tensor("B_logits", [ALPH, S], f32, kind="ExternalInput")
    bLT_t = nc.dram_tensor("B_logitsT", [S, ALPH], f32, kind="ExternalInput")
    iL_t = nc.dram_tensor("I_logits", [S], f32, kind="ExternalInput")
    out_t = nc.dram_tensor("sums", [4 * FD], f32, kind="ExternalOutput")

    with tile.TileContext(nc) as tc:
        _kernel_body(
            tc, xT_t.ap(), aL_t.ap(), bL_t.ap(), bLT_t.ap(), iL_t.ap(), out_t.ap()
        )
    nc.compile()
    return nc


def _shard_inputs(inputs, A_logits, B_logits, I_logits):
    import ml_dtypes

    in_maps = []
    bF = np.ascontiguousarray(B_logits, dtype=np.float32)
    bT = np.ascontiguousarray(B_logits.T, dtype=np.float32)
    aF = np.ascontiguousarray(A_logits, dtype=np.float32)
    iF = np.ascontiguousarray(I_logits, dtype=np.float32)
    for c in range(NC):
        xc = inputs[c * NB : (c + 1) * NB]                  # [NB, T, 32]
        # sigma-permute: column (r, chunk, b) with r = t mod C, chunk = t//C
        xs = xc.reshape(NB, K, C, ALPH).transpose(3, 2, 1, 0)   # [A, C, K, NB]
        xTc = np.ascontiguousarray(xs.reshape(ALPH, T * NB)).astype(
            ml_dtypes.bfloat16
        )
        in_maps.append(
            {
                "xT": xTc,
                "A_logits": aF,
                "B_logits": bF,
                "B_logitsT": bT,
                "I_logits": iF,
            }
        )
    return in_maps


def kernel(inputs, A_logits, B_logits, I_logits):
    from concourse.bass_utils import run_bass_kernel_spmd

    global _COMPILED
    if _COMPILED is None:
        _COMPILED = _build()

    in_maps = _shard_inputs(inputs, A_logits, B_logits, I_logits)
    res = run_bass_kernel_spmd(_COMPILED, in_maps, list(range(NC)))

    ln_corr = T * math.log(32.0) + math.log(
        np.exp(I_logits.astype(np.float64)).sum()
    )
    out = np.empty(B, np.float64)
    for c in range(NC):
        sums = np.asarray(res.results[c]["sums"], dtype=np.float64)
        sa = sums[0 : 2 * FD].reshape(K, NB)        # [chunk, seq-in-core]
        sb = sums[2 * FD : 4 * FD].reshape(K, NB)
        ll = np.log(sb).sum(0) - np.log(sa[1:]).sum(0) - ln_corr
        out[c * NB : (c + 1) * NB] = ll
    return out.astype(np.float32)
